# revision 25
# baseline (speedup 1.0000x reference)
"""GVSL loss (NCC + MSE + smoothness) as a distributed Bass kernel on 8 TRN2 cores.

Sharding: batch(2) x depth-quarters(4) = 8 shards; each core owns a 32-deep
output slab (40 input rows with the 4-voxel halo).

NCC pipeline per volume (J, I, II, JJ, IJ), fp16 throughout (fp16's 11-bit
mantissa keeps the cross-term cancellation error ~0.1% of signal):
  - W-axis 9-box via ONE DVE window-scan (op0=add, op1=subtract on a
    9-shifted view of the same padded tile)
  - H-axis 9-box + D-axis 9-box fused on the PE: 9 d-shifted fp16 band
    matmuls accumulated into one PSUM group
  - PSUM -> SBUF fp16 staging (scale 1/27 for S_I, S_J), cc math on
    [128, 4096] fp16 tiles, Ln/Exp trick, accumulated into acc columns.
"""

import os
import sys

for _p in ("/opt/trn_rl_repo",):
    if _p not in sys.path:
        sys.path.insert(0, _p)

import numpy as np
import ml_dtypes

BF16NP = ml_dtypes.bfloat16

import concourse.bass as bass
import concourse.tile as tile
from concourse import bacc, mybir
from concourse.bass_utils import run_bass_kernel_spmd

F32 = mybir.dt.float32
BF16 = mybir.dt.bfloat16
F16 = mybir.dt.float16
AF = mybir.ActivationFunctionType
ALU = mybir.AluOpType

HP = 128          # partitions (H axis)
W = 128
D_FULL = 128
DQ = 32           # output depths per core
DSLAB = 40        # input slab rows incl. +-4 halo
WPAD = 137        # 9 zeros | 128 data
LEAD = 9          # leading zeros before row 0 (for the shifted scan view)
TAIL = 4          # trailing zeros after the last row
NP_ = LEAD + DSLAB * WPAD + TAIL   # 5493 padded input length
NO = DSLAB * WPAD + TAIL           # 5484 valid scan-output length
NOA = NP_                          # alloc size for scan output views
NSTG = DQ * W                      # 4096 staged box elems
FLOW_D = DQ + 1    # 33
N_RECON = DQ * W   # 4096
N_FLOW_C = FLOW_D * W  # 4224

# acc columns
COL_CC = 0         # 4 cols (cc quarters)
COL_MSE = 4
COL_DX = 5         # +c (3 channels)
COL_DZ = 8         # +c
COL_DY = 11        # +c*8+j per psum chunk
ACC_W = 35

_CACHE = {}


def _build_program():
    nc = bacc.Bacc("TRN2", target_bir_lowering=False, debug=False, num_devices=8)

    d_J = nc.dram_tensor("J", [HP, NP_], F16, kind="ExternalInput").ap()
    d_I = nc.dram_tensor("I", [HP, NP_], F16, kind="ExternalInput").ap()
    d_recon = nc.dram_tensor("recon", [HP, N_RECON], BF16, kind="ExternalInput").ap()
    d_mseA = nc.dram_tensor("mseA", [HP, N_RECON], BF16, kind="ExternalInput").ap()
    d_flow = nc.dram_tensor("flow", [HP, 3 * N_FLOW_C], BF16, kind="ExternalInput").ap()
    d_band = nc.dram_tensor("band", [HP, HP], F16, kind="ExternalInput").ap()
    d_bidiag = nc.dram_tensor("bidiag", [HP, HP - 1], BF16, kind="ExternalInput").ap()
    d_out = nc.dram_tensor("out", [HP, ACC_W], F32, kind="ExternalOutput").ap()

    from contextlib import ExitStack

    with tile.TileContext(nc) as tc, ExitStack() as es:
        pp = es.enter_context(tc.tile_pool(name="persist", bufs=1))
        prp = es.enter_context(tc.tile_pool(name="prodp", bufs=2))
        op = es.enter_context(tc.tile_pool(name="outp", bufs=2))
        stp = es.enter_context(tc.tile_pool(name="stagep", bufs=1))
        scp = es.enter_context(tc.tile_pool(name="scrp", bufs=1))
        fp = es.enter_context(tc.tile_pool(name="flowp", bufs=1))
        fdp = es.enter_context(tc.tile_pool(name="diffp", bufs=1))
        rp = es.enter_context(tc.tile_pool(name="reconp", bufs=1))
        psp = es.enter_context(tc.tile_pool(name="psum", bufs=1, space="PSUM"))
        fpp = es.enter_context(tc.tile_pool(name="fpsum", bufs=1, space="PSUM"))

        acc = pp.tile([HP, ACC_W], F32, tag="acc", name="acc")[:]
        nc.gpsimd.memset(acc, 0.0)
        band = pp.tile([HP, HP], F16, tag="band", name="band")[:]
        bidiag = pp.tile([HP, HP - 1], BF16, tag="bidiag", name="bidiag")[:]
        PJ = pp.tile([HP, NP_], F16, tag="PJ", name="PJ")[:]
        PI = pp.tile([HP, NP_], F16, tag="PI", name="PI")[:]

        nc.sync.dma_start(out=band, in_=d_band)
        nc.sync.dma_start(out=bidiag, in_=d_bidiag)
        NJh = NP_ // 2
        nc.sync.dma_start(out=PJ[:, 0:NJh], in_=d_J[:, 0:NJh])
        nc.sync.dma_start(out=PJ[:, NJh:], in_=d_J[:, NJh:])
        nc.sync.dma_start(out=PI[:, 0:NJh], in_=d_I[:, 0:NJh])
        nc.sync.dma_start(out=PI[:, NJh:], in_=d_I[:, NJh:])

        recon = rp.tile([HP, N_RECON], BF16, tag="recon", name="recon")[:]
        mseA = rp.tile([HP, N_RECON], BF16, tag="mseA", name="mseA")[:]
        nc.sync.dma_start(out=recon, in_=d_recon)
        nc.sync.dma_start(out=mseA, in_=d_mseA)
        d_flow_r = d_flow.rearrange("p (c d w) -> p c d w", c=3, w=W)

        # data region view helper: [p, DSLAB, 128] inside a padded tile
        def data_view(t):
            rows = t[:, LEAD : LEAD + DSLAB * WPAD].rearrange(
                "p (d w) -> p d w", w=WPAD
            )
            return rows[:, :, LEAD : LEAD + W]

        # pre-zero the two product buffers (pads stay zero afterwards)
        pbufs = []
        for i in range(2):
            pb = prp.tile([HP, NP_], F16, tag="prod", name=f"pz{i}")[:]
            nc.gpsimd.memset(pb, 0.0)
            pbufs.append(pb)

        # staging tiles (box sums, fp16; J/I scaled by 1/27)
        stage = {
            v: stp.tile([HP, NSTG], F16, tag=f"st{v}", name=f"st{v}")[:]
            for v in ("J", "I", "II", "JJ", "IJ")
        }

        def ncc_volume(v, per_group=None):
            if v == "J":
                P = PJ
            elif v == "I":
                P = PI
            else:
                P = prp.tile([HP, NP_], F16, tag="prod", name="prod")[:]
                pv = data_view(P)
                if v == "II":
                    nc.scalar.activation(pv, data_view(PI), AF.Square)
                elif v == "JJ":
                    nc.scalar.activation(pv, data_view(PJ), AF.Square)
                else:
                    nc.vector.tensor_mul(pv, data_view(PI), data_view(PJ))

            # W-axis 9-window sums via a single scan pass (two chained halves)
            O = op.tile([HP, NOA], F16, tag="obox", name="obox")[:]
            NH = (DSLAB // 2) * WPAD  # 2740, row-20 boundary
            nc.vector.tensor_tensor_scan(
                O[:, 0:NH], P[:, LEAD : LEAD + NH], P[:, 0:NH],
                0.0, op0=ALU.add, op1=ALU.subtract,
            )
            nc.vector.tensor_tensor_scan(
                O[:, NH:NO], P[:, LEAD + NH : NP_], P[:, NH:NO],
                O[:, NH - 1 : NH], op0=ALU.add, op1=ALU.subtract,
            )

            # H-box + D-box on PE: 9 d-shifted band matmuls into one psum group
            # shift-j view: rows r=j.. at 137-stride, out col 13+137r+w
            Vs = [
                O[:, 13 + WPAD * j : 13 + WPAD * j + WPAD * DQ].rearrange(
                    "p (d w) -> p d w", w=WPAD
                )
                for j in range(9)
            ]
            sc = 1.0 / 27.0 if v in ("J", "I") else 1.0
            for g in range(4):  # 8 output depths per group, 2 psum halves
                ps = psp.tile([HP, 1024], F32, tag="ps", name="ps", bufs=2)[:]
                for h in range(2):
                    r0 = 8 * g + 4 * h
                    for j in range(9):
                        nc.tensor.matmul(
                            ps[:, 512 * h : 512 * (h + 1)],
                            band,
                            Vs[j][:, r0 : r0 + 4, 0:W],
                            start=(j == 0),
                            stop=(j == 8),
                        )
                nc.scalar.activation(
                    stage[v][:, 1024 * g : 1024 * (g + 1)], ps,
                    AF.Copy, scale=sc,
                )
                if per_group is not None:
                    per_group(g)

        eps_ap = pp.tile([HP, 1], F32, tag="epsc", name="epsc")[:]
        nc.gpsimd.memset(eps_ap, 1e-5)
        # persistent cc intermediates; m1_t is reused as Ivar storage after
        # the cross phase consumes a_I*a_J
        m1_t = pp.tile([HP, NSTG], F16, tag="m1t", name="m1t")[:]
        lncr_t = pp.tile([HP, NSTG], F16, tag="lncrt", name="lncrt")[:]

        def cc_m1(half):
            # a_I*a_J — needs J, I stages only
            lo, hi = half * (NSTG // 2), (half + 1) * (NSTG // 2)
            nc.vector.tensor_mul(
                m1_t[:, lo:hi], stage["I"][:, lo:hi], stage["J"][:, lo:hi]
            )

        def cc_cross(q):
            # ln|cross| per 1024-col quarter, gated on IJ MM group q
            NQ = NSTG // 4
            lo, hi = q * NQ, (q + 1) * NQ
            s1 = scp.tile([HP, NQ], F16, tag=f"q1{q % 2}", name=f"q1{q % 2}")[:]
            s2 = scp.tile([HP, NQ], F16, tag=f"q2{q % 2}", name=f"q2{q % 2}")[:]
            nc.vector.tensor_sub(s2, stage["IJ"][:, lo:hi], m1_t[:, lo:hi])
            nc.vector.scalar_tensor_tensor(
                s1, s2, -1.0, s2, op0=ALU.mult, op1=ALU.max
            )                                                 # |cross|
            nc.scalar.activation(lncr_t[:, lo:hi], s1, AF.Ln)  # ln|cross|

        def cc_ivar(half):
            # I_var into m1_t (dead after cross phase) — needs II stage
            lo, hi = half * (NSTG // 2), (half + 1) * (NSTG // 2)
            NH2 = NSTG // 2
            s3 = scp.tile([HP, NH2], F16, tag=f"h3{half}", name=f"h3{half}")[:]
            nc.scalar.activation(s3, stage["I"][:, lo:hi], AF.Square)
            nc.vector.tensor_sub(m1_t[:, lo:hi], stage["II"][:, lo:hi], s3)

        def cc_final(q):
            # J_var, denom, ln, exp per quarter, gated on JJ MM group q
            NQ = NSTG // 4
            lo, hi = q * NQ, (q + 1) * NQ
            s1 = scp.tile([HP, NQ], F16, tag=f"q1{q % 2}", name=f"q1{q % 2}")[:]
            s2 = scp.tile([HP, NQ], F16, tag=f"q2{q % 2}", name=f"q2{q % 2}")[:]
            nc.scalar.activation(s1, stage["J"][:, lo:hi], AF.Square)
            nc.vector.tensor_sub(s2, stage["JJ"][:, lo:hi], s1)   # J_var
            nc.vector.tensor_mul(s1, s2, m1_t[:, lo:hi])          # Ivar*Jvar
            nc.scalar.activation(s2, s1, AF.Ln, bias=eps_ap)      # ln(den+eps)
            nc.vector.scalar_tensor_tensor(
                s1, lncr_t[:, lo:hi], 2.0, s2, op0=ALU.mult, op1=ALU.subtract
            )                                                 # 2ln|cross|-ln(den)
            nc.scalar.activation(
                s2, s1, AF.Exp,
                accum_out=acc[:, COL_CC + q : COL_CC + q + 1],
            )

        def mse_compute():
            mbuf = fdp.tile([HP, N_RECON], BF16, tag="dbuf", name="dbuf")[:]
            nc.vector.tensor_sub(mbuf, mseA, recon)
            nc.scalar.activation(
                mbuf, mbuf, AF.Square, accum_out=acc[:, COL_MSE : COL_MSE + 1]
            )

        flow_tiles = []

        def flow_dma(c):
            fc = fp.tile([HP, N_FLOW_C], BF16, tag=f"fc{c}", name=f"fc{c}")[:]
            nc.sync.dma_start(
                out=fc, in_=d_flow_r[:, c].rearrange("p d w -> p (d w)")
            )
            flow_tiles.append(fc)

        def flow_dy(c):
            # H-axis diffs on the PE: psum = bidiag^T @ fc, squared on PSUM
            fc_r = flow_tiles[c].rearrange("p (d w) -> p d w", w=W)
            fc_flat = fc_r[:, 0:DQ, :].rearrange("p d w -> p (d w)")
            for j in range(N_RECON // 1024):
                ps = fpp.tile([HP, 1024], F32, tag="fps", name="fps", bufs=2)[:]
                for h in range(2):
                    o0 = 1024 * j + 512 * h
                    nc.tensor.matmul(
                        ps[0 : HP - 1, 512 * h : 512 * (h + 1)],
                        bidiag,
                        fc_flat[:, o0 : o0 + 512],
                        start=True,
                        stop=True,
                    )
                col = COL_DY + c * 8 + j
                nc.scalar.activation(
                    ps[0 : HP - 1, :],
                    ps[0 : HP - 1, :],
                    AF.Square,
                    accum_out=acc[0 : HP - 1, col : col + 1],
                )

        def flow_dxdz(c):
            fc_r = flow_tiles[c].rearrange("p (d w) -> p d w", w=W)

            # W-axis diffs (innermost)
            db = fdp.tile([HP, N_RECON], BF16, tag="dbuf", name="dbuf")[:]
            db_x = db.rearrange("p (d w) -> p d w", w=W)[:, :, 0 : W - 1]
            nc.vector.tensor_sub(
                db_x, fc_r[:, 0:DQ, 1:W], fc_r[:, 0:DQ, 0 : W - 1]
            )
            col = COL_DX + c
            nc.scalar.activation(
                db.rearrange("p (d w) -> p d w", w=W)[:, :, 0 : W - 1],
                db.rearrange("p (d w) -> p d w", w=W)[:, :, 0 : W - 1],
                AF.Square,
                accum_out=acc[:, col : col + 1],
            )

            # D-axis diffs
            db = fdp.tile([HP, N_RECON], BF16, tag="dbuf", name="dbuf")[:]
            nc.vector.tensor_sub(
                db,
                fc_r[:, 1 : DQ + 1, :].rearrange("p d w -> p (d w)"),
                fc_r[:, 0:DQ, :].rearrange("p d w -> p (d w)"),
            )
            col = COL_DZ + c
            nc.scalar.activation(
                db, db, AF.Square, accum_out=acc[:, col : col + 1]
            )

        # schedule: flow DMAs first; flow-dy (PE+ACT only) hoisted early to
        # warm the PE while the DVE runs the first scans; volume order
        # J, I, IJ, II, JJ so the cross chain runs mid-kernel and only the
        # light Jvar/denominator chain (per JJ MM group) trails the end
        for c in range(3):
            flow_dma(c)
        flow_dy(0)
        ncc_volume("J")
        flow_dy(1)
        ncc_volume("I")
        flow_dy(2)
        cc_m1(0)
        cc_m1(1)
        ncc_volume("IJ", per_group=cc_cross)
        flow_dxdz(0)
        ncc_volume("II")
        flow_dxdz(1)
        cc_ivar(0)
        cc_ivar(1)
        flow_dxdz(2)
        mse_compute()
        ncc_volume("JJ", per_group=cc_final)
        nc.sync.dma_start(out=d_out, in_=acc)

    nc.compile()
    return nc


def _make_consts() -> tuple[np.ndarray, np.ndarray]:
    k = np.arange(HP)
    band = (np.abs(k[:, None] - k[None, :]) <= 4).astype(np.float16)
    m = np.arange(HP - 1)
    bidiag = np.zeros((HP, HP - 1), BF16NP)
    bidiag[m + 1, m] = 1.0
    bidiag[m, m] = -1.0
    return band, bidiag


def _shard_inputs(imgsA, recon_A, warped_BA, flow_BA):
    band, bidiag = _make_consts()
    in_maps = []
    for core in range(8):
        b, q = divmod(core, 4)
        d0 = DQ * q

        def slab(vol):
            # padded fp16 slab: LEAD zeros, DSLAB rows of [9 zeros|128 data],
            # TAIL zeros; volume centered by -0.5
            s = np.zeros((HP, NP_), np.float16)
            rows = s[:, LEAD : LEAD + DSLAB * WPAD].reshape(HP, DSLAB, WPAD)
            lo, hi = d0 - 4, d0 + DQ + 4
            clo, chi = max(lo, 0), min(hi, D_FULL)
            rows[:, clo - lo : chi - lo, LEAD : LEAD + W] = np.ascontiguousarray(
                vol[clo:chi].transpose(1, 0, 2)
            ).astype(np.float16)
            return s

        rec = np.ascontiguousarray(
            recon_A[b, 0, d0 : d0 + DQ].transpose(1, 0, 2)
        ).astype(BF16NP).reshape(HP, N_RECON)
        msea = np.ascontiguousarray(
            imgsA[b, 0, d0 : d0 + DQ].transpose(1, 0, 2)
        ).astype(BF16NP).reshape(HP, N_RECON)

        fl = np.empty((HP, 3, FLOW_D, W), BF16NP)
        hi = min(d0 + FLOW_D, D_FULL)
        n = hi - d0
        fl[:, :, :n] = flow_BA[b, :, d0:hi].transpose(2, 0, 1, 3)
        if n < FLOW_D:
            fl[:, :, n:] = fl[:, :, n - 1 : n]

        in_maps.append(
            {
                "J": slab(imgsA[b, 0]),
                "I": slab(warped_BA[b, 0]),
                "recon": rec,
                "mseA": msea,
                "flow": np.ascontiguousarray(fl).reshape(HP, 3 * N_FLOW_C),
                "band": band,
                "bidiag": bidiag,
            }
        )
    return in_maps


def _install_profile_shim():
    """Wire up NTFF profiling under axon when antenv.axon_hooks is absent."""
    try:
        import antenv.axon_hooks  # noqa: F401

        return True
    except ImportError:
        pass
    import contextlib
    import ctypes
    import types

    so_path = "/opt/axon/libaxon_pjrt.so"
    if not os.path.exists(so_path):
        return False
    lib = ctypes.CDLL(so_path)
    if not hasattr(lib, "axon_start_nrt_profile"):
        return False
    lib.axon_start_nrt_profile.argtypes = [
        ctypes.POINTER(ctypes.c_int64),
        ctypes.c_size_t,
    ]
    lib.axon_start_nrt_profile.restype = ctypes.c_int64
    lib.axon_stop_nrt_profile.argtypes = [ctypes.c_char_p]
    lib.axon_stop_nrt_profile.restype = ctypes.c_int64

    @contextlib.contextmanager
    def _hook(output_dir, device_ids):
        import jax

        jax.devices()
        if device_ids:
            ids = (ctypes.c_int64 * len(device_ids))(*device_ids)
            rc = lib.axon_start_nrt_profile(ids, len(device_ids))
        else:
            rc = lib.axon_start_nrt_profile(None, 0)
        if rc != 0:
            raise RuntimeError(f"axon_start_nrt_profile rc={rc}")
        try:
            yield
        finally:
            n = lib.axon_stop_nrt_profile(str(output_dir).encode())
            print(f"ntff profile: {n} file(s) written to {output_dir}")

    mod = types.ModuleType("antenv.axon_hooks")
    mod.get_axon_ntff_profile_hook = lambda: _hook
    mod.set_axon_ntff_profile_hook = lambda h: None
    import antenv

    sys.modules["antenv.axon_hooks"] = mod
    antenv.axon_hooks = mod

    # keep profile artifacts local instead of uploading to fishnet
    import concourse.bass_utils as _bu

    _bu.upload_artifacts = lambda tmpdir: tmpdir
    return True


LAST_EXEC_NS = None
LAST_RESULTS = None


def kernel(imgsA, recon_A, warped_BA, flow_BA):
    global LAST_EXEC_NS, LAST_RESULTS
    if "nc" not in _CACHE:
        _CACHE["nc"] = _build_program()
    nc = _CACHE["nc"]

    in_maps = _shard_inputs(
        np.asarray(imgsA, np.float32),
        np.asarray(recon_A, np.float32),
        np.asarray(warped_BA, np.float32),
        np.asarray(flow_BA, np.float32),
    )
    trace = os.environ.get("GVSL_TRACE", "0") == "1"
    if trace:
        trace = _install_profile_shim()
    tmpdir = os.environ.get("GVSL_TRACE_DIR") or None
    res = run_bass_kernel_spmd(
        nc, in_maps, core_ids=list(range(8)), trace=trace, tmpdir=tmpdir
    )
    LAST_EXEC_NS = res.exec_time_ns
    LAST_RESULTS = res

    cc = mse = dx = dy = dz = 0.0
    for r in res.results:
        o = np.asarray(r["out"], np.float64)
        cc += o[:, COL_CC : COL_CC + 4].sum()
        mse += o[:, COL_MSE].sum()
        dx += o[:, COL_DX : COL_DX + 3].sum()
        dy += o[: HP - 1, COL_DY : COL_DY + 24].sum()
        dz += o[:, COL_DZ : COL_DZ + 3].sum()

    n_vox = 2 * 1 * 128 * 128 * 128
    n_d = 2 * 3 * 127 * 128 * 128
    ncc_loss = 1.0 - cc / n_vox
    mse_loss = mse / n_vox
    smooth_loss = (dx / n_d + dy / n_d + dz / n_d) / 3.0
    return (
        np.float32(ncc_loss),
        np.float32(mse_loss),
        np.float32(smooth_loss),
    )


# revision 30
# speedup vs baseline: 1.0930x; 1.0930x over previous
"""GVSL loss (NCC + MSE + smoothness) as a distributed Bass kernel on 8 TRN2 cores.

Sharding: batch(2) x depth-quarters(4) = 8 shards; each core owns a 32-deep
output slab (40 input rows with the 4-voxel halo).

NCC pipeline per volume (J, I, II, JJ, IJ), fp16 throughout (fp16's 11-bit
mantissa keeps the cross-term cancellation error ~0.1% of signal):
  - W-axis 9-box via ONE DVE window-scan (op0=add, op1=subtract on a
    9-shifted view of the same padded tile)
  - H-axis 9-box + D-axis 9-box fused on the PE: 9 d-shifted fp16 band
    matmuls accumulated into one PSUM group
  - PSUM -> SBUF fp16 staging (scale 1/27 for S_I, S_J), cc math on
    [128, 4096] fp16 tiles, Ln/Exp trick, accumulated into acc columns.
"""

import os
import sys

for _p in ("/opt/trn_rl_repo",):
    if _p not in sys.path:
        sys.path.insert(0, _p)

import numpy as np
import ml_dtypes

BF16NP = ml_dtypes.bfloat16

import concourse.bass as bass
import concourse.tile as tile
from concourse import bacc, mybir
from concourse.bass_utils import run_bass_kernel_spmd

F32 = mybir.dt.float32
BF16 = mybir.dt.bfloat16
F16 = mybir.dt.float16
AF = mybir.ActivationFunctionType
ALU = mybir.AluOpType

HP = 128          # partitions (H axis)
W = 128
D_FULL = 128
DQ = 32           # output depths per core
DSLAB = 40        # input slab rows incl. +-4 halo
WPAD = 137        # 9 zeros | 128 data
LEAD = 9          # leading zeros before row 0 (for the shifted scan view)
TAIL = 4          # trailing zeros after the last row
NP_ = LEAD + DSLAB * WPAD + TAIL   # 5493 padded input length
NO = DSLAB * WPAD + TAIL           # 5484 valid scan-output length
NOA = NP_                          # alloc size for scan output views
NSTG = DQ * W                      # 4096 staged box elems
FLOW_D = DQ + 1    # 33
N_RECON = DQ * W   # 4096
N_FLOW_C = FLOW_D * W  # 4224

# acc columns
COL_CC = 0         # 4 cols (cc quarters)
COL_MSE = 4
COL_DX = 5         # +c (3 channels)
COL_DZ = 8         # +c
COL_DY = 11        # +c*8+j per psum chunk
ACC_W = 35

_CACHE = {}


def _build_program():
    nc = bacc.Bacc("TRN2", target_bir_lowering=False, debug=False, num_devices=8)

    d_J = nc.dram_tensor("J", [HP, NP_], F16, kind="ExternalInput").ap()
    d_I = nc.dram_tensor("I", [HP, NP_], F16, kind="ExternalInput").ap()
    d_recon = nc.dram_tensor("recon", [HP, N_RECON], BF16, kind="ExternalInput").ap()
    d_mseA = nc.dram_tensor("mseA", [HP, N_RECON], BF16, kind="ExternalInput").ap()
    d_flow = nc.dram_tensor("flow", [HP, 3 * N_FLOW_C], BF16, kind="ExternalInput").ap()
    d_band = nc.dram_tensor("band", [HP, HP], F16, kind="ExternalInput").ap()
    d_bidiag = nc.dram_tensor("bidiag", [HP, HP - 1], BF16, kind="ExternalInput").ap()
    d_out = nc.dram_tensor("out", [HP, ACC_W], F32, kind="ExternalOutput").ap()

    from contextlib import ExitStack

    with tile.TileContext(nc) as tc, ExitStack() as es:
        pp = es.enter_context(tc.tile_pool(name="persist", bufs=1))
        prp = es.enter_context(tc.tile_pool(name="prodp", bufs=3))
        op = es.enter_context(tc.tile_pool(name="outp", bufs=2))
        stp = es.enter_context(tc.tile_pool(name="stagep", bufs=1))
        scp = es.enter_context(tc.tile_pool(name="scrp", bufs=1))
        fp = es.enter_context(tc.tile_pool(name="flowp", bufs=1))
        fdp = es.enter_context(tc.tile_pool(name="diffp", bufs=1))
        rp = es.enter_context(tc.tile_pool(name="reconp", bufs=1))
        psp = es.enter_context(tc.tile_pool(name="psum", bufs=1, space="PSUM"))
        fpp = es.enter_context(tc.tile_pool(name="fpsum", bufs=1, space="PSUM"))

        acc = pp.tile([HP, ACC_W], F32, tag="acc", name="acc")[:]
        nc.gpsimd.memset(acc, 0.0)
        band = pp.tile([HP, HP], F16, tag="band", name="band")[:]
        bidiag = pp.tile([HP, HP - 1], BF16, tag="bidiag", name="bidiag")[:]
        PJ = pp.tile([HP, NP_], F16, tag="PJ", name="PJ")[:]
        PI = pp.tile([HP, NP_], F16, tag="PI", name="PI")[:]

        nc.sync.dma_start(out=band, in_=d_band)
        nc.sync.dma_start(out=bidiag, in_=d_bidiag)
        # J in quarters (earliest possible first-scan start), I in halves
        NJq = LEAD + 10 * WPAD  # 1379, row-10 boundary
        cuts = [0, NJq, LEAD + 20 * WPAD, LEAD + 30 * WPAD, NP_]
        for a, b in zip(cuts[:-1], cuts[1:]):
            nc.sync.dma_start(out=PJ[:, a:b], in_=d_J[:, a:b])
        NJh = NP_ // 2
        nc.sync.dma_start(out=PI[:, 0:NJh], in_=d_I[:, 0:NJh])
        nc.sync.dma_start(out=PI[:, NJh:], in_=d_I[:, NJh:])

        recon = rp.tile([HP, N_RECON], BF16, tag="recon", name="recon")[:]
        mseA = rp.tile([HP, N_RECON], BF16, tag="mseA", name="mseA")[:]
        nc.sync.dma_start(out=recon, in_=d_recon)
        nc.sync.dma_start(out=mseA, in_=d_mseA)
        d_flow_r = d_flow.rearrange("p (c d w) -> p c d w", c=3, w=W)

        # data region view helper: [p, DSLAB, 128] inside a padded tile
        def data_view(t):
            rows = t[:, LEAD : LEAD + DSLAB * WPAD].rearrange(
                "p (d w) -> p d w", w=WPAD
            )
            return rows[:, :, LEAD : LEAD + W]

        # pre-zero the three product buffers (pads stay zero afterwards)
        pbufs = []
        for i in range(3):
            pb = prp.tile([HP, NP_], F16, tag="prod", name=f"pz{i}")[:]
            nc.gpsimd.memset(pb, 0.0)
            pbufs.append(pb)

        # staging tiles (box sums, fp16; J/I scaled by 1/27)
        stage = {
            v: stp.tile([HP, NSTG], F16, tag=f"st{v}", name=f"st{v}")[:]
            for v in ("J", "I", "II", "JJ", "IJ")
        }

        prod_tiles = {}

        def make_product(v):
            P = prp.tile([HP, NP_], F16, tag="prod", name="prod")[:]
            pv = data_view(P)
            if v == "II":
                nc.scalar.activation(pv, data_view(PI), AF.Square)
            elif v == "JJ":
                nc.scalar.activation(pv, data_view(PJ), AF.Square)
            else:
                nc.vector.tensor_mul(pv, data_view(PI), data_view(PJ))
            prod_tiles[v] = P

        def ncc_volume(v, per_group=None, nsplit=2):
            if v == "J":
                P = PJ
            elif v == "I":
                P = PI
            else:
                P = prod_tiles[v]

            # W-axis 9-window sums via chained window-scan pieces
            O = op.tile([HP, NOA], F16, tag="obox", name="obox")[:]
            rows_per = DSLAB // nsplit
            for k in range(nsplit):
                a = k * rows_per * WPAD
                b = NO if k == nsplit - 1 else (k + 1) * rows_per * WPAD
                init = 0.0 if k == 0 else O[:, a - 1 : a]
                nc.vector.tensor_tensor_scan(
                    O[:, a:b], P[:, LEAD + a : LEAD + b], P[:, a:b],
                    init, op0=ALU.add, op1=ALU.subtract,
                )

            # H-box + D-box on PE: 9 d-shifted band matmuls into one psum group
            # shift-j view: rows r=j.. at 137-stride, out col 13+137r+w
            Vs = [
                O[:, 13 + WPAD * j : 13 + WPAD * j + WPAD * DQ].rearrange(
                    "p (d w) -> p d w", w=WPAD
                )
                for j in range(9)
            ]
            sc = 1.0 / 27.0 if v in ("J", "I") else 1.0
            for g in range(4):  # 8 output depths per group, 2 psum halves
                ps = psp.tile([HP, 1024], F32, tag="ps", name="ps", bufs=2)[:]
                for h in range(2):
                    r0 = 8 * g + 4 * h
                    for j in range(9):
                        nc.tensor.matmul(
                            ps[:, 512 * h : 512 * (h + 1)],
                            band,
                            Vs[j][:, r0 : r0 + 4, 0:W],
                            start=(j == 0),
                            stop=(j == 8),
                        )
                nc.scalar.activation(
                    stage[v][:, 1024 * g : 1024 * (g + 1)], ps,
                    AF.Copy, scale=sc,
                )
                if per_group is not None:
                    per_group(g)

        eps_ap = pp.tile([HP, 1], F32, tag="epsc", name="epsc")[:]
        nc.gpsimd.memset(eps_ap, 1e-5)
        # persistent cc intermediates; m1_t is reused as Ivar storage after
        # the cross phase consumes a_I*a_J
        m1_t = pp.tile([HP, NSTG], F16, tag="m1t", name="m1t")[:]
        lncr_t = pp.tile([HP, NSTG], F16, tag="lncrt", name="lncrt")[:]

        def cc_m1(half):
            # a_I*a_J — needs J, I stages only
            lo, hi = half * (NSTG // 2), (half + 1) * (NSTG // 2)
            nc.vector.tensor_mul(
                m1_t[:, lo:hi], stage["I"][:, lo:hi], stage["J"][:, lo:hi]
            )

        def cc_cross(q):
            # ln|cross| per 1024-col quarter, gated on IJ MM group q
            NQ = NSTG // 4
            lo, hi = q * NQ, (q + 1) * NQ
            s1 = scp.tile([HP, NQ], F16, tag=f"q1{q % 2}", name=f"q1{q % 2}")[:]
            s2 = scp.tile([HP, NQ], F16, tag=f"q2{q % 2}", name=f"q2{q % 2}")[:]
            nc.vector.tensor_sub(s2, stage["IJ"][:, lo:hi], m1_t[:, lo:hi])
            nc.vector.scalar_tensor_tensor(
                s1, s2, -1.0, s2, op0=ALU.mult, op1=ALU.max
            )                                                 # |cross|
            nc.scalar.activation(lncr_t[:, lo:hi], s1, AF.Ln)  # ln|cross|

        def cc_ivar(half):
            # I_var into m1_t (dead after cross phase) — needs II stage
            lo, hi = half * (NSTG // 2), (half + 1) * (NSTG // 2)
            NH2 = NSTG // 2
            s3 = scp.tile([HP, NH2], F16, tag=f"h3{half}", name=f"h3{half}")[:]
            nc.scalar.activation(s3, stage["I"][:, lo:hi], AF.Square)
            nc.vector.tensor_sub(m1_t[:, lo:hi], stage["II"][:, lo:hi], s3)

        def cc_final(q):
            # J_var, denom, ln, exp per quarter, gated on JJ MM group q
            NQ = NSTG // 4
            lo, hi = q * NQ, (q + 1) * NQ
            s1 = scp.tile([HP, NQ], F16, tag=f"q1{q % 2}", name=f"q1{q % 2}")[:]
            s2 = scp.tile([HP, NQ], F16, tag=f"q2{q % 2}", name=f"q2{q % 2}")[:]
            nc.scalar.activation(s1, stage["J"][:, lo:hi], AF.Square)
            nc.vector.tensor_sub(s2, stage["JJ"][:, lo:hi], s1)   # J_var
            nc.vector.tensor_mul(s1, s2, m1_t[:, lo:hi])          # Ivar*Jvar
            nc.scalar.activation(s2, s1, AF.Ln, bias=eps_ap)      # ln(den+eps)
            nc.vector.scalar_tensor_tensor(
                s1, lncr_t[:, lo:hi], 2.0, s2, op0=ALU.mult, op1=ALU.subtract
            )                                                 # 2ln|cross|-ln(den)
            nc.scalar.activation(
                s2, s1, AF.Exp,
                accum_out=acc[:, COL_CC + q : COL_CC + q + 1],
            )

        def mse_compute():
            mbuf = fdp.tile([HP, N_RECON], BF16, tag="dbuf", name="dbuf")[:]
            nc.vector.tensor_sub(mbuf, mseA, recon)
            nc.scalar.activation(
                mbuf, mbuf, AF.Square, accum_out=acc[:, COL_MSE : COL_MSE + 1]
            )

        flow_tiles = []

        def flow_dma(c):
            fc = fp.tile([HP, N_FLOW_C], BF16, tag=f"fc{c}", name=f"fc{c}")[:]
            nc.sync.dma_start(
                out=fc, in_=d_flow_r[:, c].rearrange("p d w -> p (d w)")
            )
            flow_tiles.append(fc)

        def flow_dy(c):
            # H-axis diffs on the PE: psum = bidiag^T @ fc, squared on PSUM
            fc_r = flow_tiles[c].rearrange("p (d w) -> p d w", w=W)
            fc_flat = fc_r[:, 0:DQ, :].rearrange("p d w -> p (d w)")
            for j in range(N_RECON // 1024):
                ps = fpp.tile([HP, 1024], F32, tag="fps", name="fps", bufs=2)[:]
                for h in range(2):
                    o0 = 1024 * j + 512 * h
                    nc.tensor.matmul(
                        ps[0 : HP - 1, 512 * h : 512 * (h + 1)],
                        bidiag,
                        fc_flat[:, o0 : o0 + 512],
                        start=True,
                        stop=True,
                    )
                col = COL_DY + c * 8 + j
                nc.scalar.activation(
                    ps[0 : HP - 1, :],
                    ps[0 : HP - 1, :],
                    AF.Square,
                    accum_out=acc[0 : HP - 1, col : col + 1],
                )

        def flow_dxdz(c):
            fc_r = flow_tiles[c].rearrange("p (d w) -> p d w", w=W)

            # W-axis diffs (innermost)
            db = fdp.tile([HP, N_RECON], BF16, tag="dbuf", name="dbuf")[:]
            db_x = db.rearrange("p (d w) -> p d w", w=W)[:, :, 0 : W - 1]
            nc.vector.tensor_sub(
                db_x, fc_r[:, 0:DQ, 1:W], fc_r[:, 0:DQ, 0 : W - 1]
            )
            col = COL_DX + c
            nc.scalar.activation(
                db.rearrange("p (d w) -> p d w", w=W)[:, :, 0 : W - 1],
                db.rearrange("p (d w) -> p d w", w=W)[:, :, 0 : W - 1],
                AF.Square,
                accum_out=acc[:, col : col + 1],
            )

            # D-axis diffs
            db = fdp.tile([HP, N_RECON], BF16, tag="dbuf", name="dbuf")[:]
            nc.vector.tensor_sub(
                db,
                fc_r[:, 1 : DQ + 1, :].rearrange("p d w -> p (d w)"),
                fc_r[:, 0:DQ, :].rearrange("p d w -> p (d w)"),
            )
            col = COL_DZ + c
            nc.scalar.activation(
                db, db, AF.Square, accum_out=acc[:, col : col + 1]
            )

        # schedule: flow DMAs first; flow-dy (PE+ACT only) hoisted early to
        # warm the PE while the DVE runs the first scans; volume order
        # J, I, IJ, II, JJ so the cross chain runs mid-kernel and only the
        # light Jvar/denominator chain (per JJ MM group) trails the end
        for c in range(3):
            flow_dma(c)
        flow_dy(0)
        ncc_volume("J", nsplit=4)
        make_product("II")   # ACT squares issued early (3 product buffers,
        make_product("JJ")   # no WAR hazard) so they never gate the scans
        flow_dy(1)
        ncc_volume("I")
        flow_dy(2)
        cc_m1(0)
        cc_m1(1)
        make_product("IJ")
        ncc_volume("IJ", per_group=cc_cross)
        flow_dxdz(0)
        ncc_volume("II")
        flow_dxdz(1)
        cc_ivar(0)
        cc_ivar(1)
        flow_dxdz(2)
        mse_compute()
        ncc_volume("JJ", per_group=cc_final)
        nc.sync.dma_start(out=d_out, in_=acc)

    nc.compile()
    return nc


def _make_consts() -> tuple[np.ndarray, np.ndarray]:
    k = np.arange(HP)
    band = (np.abs(k[:, None] - k[None, :]) <= 4).astype(np.float16)
    m = np.arange(HP - 1)
    bidiag = np.zeros((HP, HP - 1), BF16NP)
    bidiag[m + 1, m] = 1.0
    bidiag[m, m] = -1.0
    return band, bidiag


def _shard_inputs(imgsA, recon_A, warped_BA, flow_BA):
    band, bidiag = _make_consts()
    in_maps = []
    for core in range(8):
        b, q = divmod(core, 4)
        d0 = DQ * q

        def slab(vol):
            # padded fp16 slab: LEAD zeros, DSLAB rows of [9 zeros|128 data],
            # TAIL zeros; volume centered by -0.5
            s = np.zeros((HP, NP_), np.float16)
            rows = s[:, LEAD : LEAD + DSLAB * WPAD].reshape(HP, DSLAB, WPAD)
            lo, hi = d0 - 4, d0 + DQ + 4
            clo, chi = max(lo, 0), min(hi, D_FULL)
            rows[:, clo - lo : chi - lo, LEAD : LEAD + W] = np.ascontiguousarray(
                vol[clo:chi].transpose(1, 0, 2)
            ).astype(np.float16)
            return s

        rec = np.ascontiguousarray(
            recon_A[b, 0, d0 : d0 + DQ].transpose(1, 0, 2)
        ).astype(BF16NP).reshape(HP, N_RECON)
        msea = np.ascontiguousarray(
            imgsA[b, 0, d0 : d0 + DQ].transpose(1, 0, 2)
        ).astype(BF16NP).reshape(HP, N_RECON)

        fl = np.empty((HP, 3, FLOW_D, W), BF16NP)
        hi = min(d0 + FLOW_D, D_FULL)
        n = hi - d0
        fl[:, :, :n] = flow_BA[b, :, d0:hi].transpose(2, 0, 1, 3)
        if n < FLOW_D:
            fl[:, :, n:] = fl[:, :, n - 1 : n]

        in_maps.append(
            {
                "J": slab(imgsA[b, 0]),
                "I": slab(warped_BA[b, 0]),
                "recon": rec,
                "mseA": msea,
                "flow": np.ascontiguousarray(fl).reshape(HP, 3 * N_FLOW_C),
                "band": band,
                "bidiag": bidiag,
            }
        )
    return in_maps


def _install_profile_shim():
    """Wire up NTFF profiling under axon when antenv.axon_hooks is absent."""
    try:
        import antenv.axon_hooks  # noqa: F401

        return True
    except ImportError:
        pass
    import contextlib
    import ctypes
    import types

    so_path = "/opt/axon/libaxon_pjrt.so"
    if not os.path.exists(so_path):
        return False
    lib = ctypes.CDLL(so_path)
    if not hasattr(lib, "axon_start_nrt_profile"):
        return False
    lib.axon_start_nrt_profile.argtypes = [
        ctypes.POINTER(ctypes.c_int64),
        ctypes.c_size_t,
    ]
    lib.axon_start_nrt_profile.restype = ctypes.c_int64
    lib.axon_stop_nrt_profile.argtypes = [ctypes.c_char_p]
    lib.axon_stop_nrt_profile.restype = ctypes.c_int64

    @contextlib.contextmanager
    def _hook(output_dir, device_ids):
        import jax

        jax.devices()
        if device_ids:
            ids = (ctypes.c_int64 * len(device_ids))(*device_ids)
            rc = lib.axon_start_nrt_profile(ids, len(device_ids))
        else:
            rc = lib.axon_start_nrt_profile(None, 0)
        if rc != 0:
            raise RuntimeError(f"axon_start_nrt_profile rc={rc}")
        try:
            yield
        finally:
            n = lib.axon_stop_nrt_profile(str(output_dir).encode())
            print(f"ntff profile: {n} file(s) written to {output_dir}")

    mod = types.ModuleType("antenv.axon_hooks")
    mod.get_axon_ntff_profile_hook = lambda: _hook
    mod.set_axon_ntff_profile_hook = lambda h: None
    import antenv

    sys.modules["antenv.axon_hooks"] = mod
    antenv.axon_hooks = mod

    # keep profile artifacts local instead of uploading to fishnet
    import concourse.bass_utils as _bu

    _bu.upload_artifacts = lambda tmpdir: tmpdir
    return True


LAST_EXEC_NS = None
LAST_RESULTS = None


def kernel(imgsA, recon_A, warped_BA, flow_BA):
    global LAST_EXEC_NS, LAST_RESULTS
    if "nc" not in _CACHE:
        _CACHE["nc"] = _build_program()
    nc = _CACHE["nc"]

    in_maps = _shard_inputs(
        np.asarray(imgsA, np.float32),
        np.asarray(recon_A, np.float32),
        np.asarray(warped_BA, np.float32),
        np.asarray(flow_BA, np.float32),
    )
    trace = os.environ.get("GVSL_TRACE", "0") == "1"
    if trace:
        trace = _install_profile_shim()
    tmpdir = os.environ.get("GVSL_TRACE_DIR") or None
    res = run_bass_kernel_spmd(
        nc, in_maps, core_ids=list(range(8)), trace=trace, tmpdir=tmpdir
    )
    LAST_EXEC_NS = res.exec_time_ns
    LAST_RESULTS = res

    cc = mse = dx = dy = dz = 0.0
    for r in res.results:
        o = np.asarray(r["out"], np.float64)
        cc += o[:, COL_CC : COL_CC + 4].sum()
        mse += o[:, COL_MSE].sum()
        dx += o[:, COL_DX : COL_DX + 3].sum()
        dy += o[: HP - 1, COL_DY : COL_DY + 24].sum()
        dz += o[:, COL_DZ : COL_DZ + 3].sum()

    n_vox = 2 * 1 * 128 * 128 * 128
    n_d = 2 * 3 * 127 * 128 * 128
    ncc_loss = 1.0 - cc / n_vox
    mse_loss = mse / n_vox
    smooth_loss = (dx / n_d + dy / n_d + dz / n_d) / 3.0
    return (
        np.float32(ncc_loss),
        np.float32(mse_loss),
        np.float32(smooth_loss),
    )
